# revision 16
# baseline (speedup 1.0000x reference)
"""Trainium2 Bass kernel for nn_EnvironmentConditionalTransformer.

Contract: kernel(**inputs) takes the FULL unsharded inputs (numpy arrays,
keys as in setup_inputs()) and returns the FULL [B, G] float32 output.

Sharding: 8 cores = batch(4) x gene-halves(2). Each core runs the SNP
encoder for its batch over all S=4096 SNPs, projects K/V, and computes
masked attention + FiLM + head for its 512 genes. Zero collectives; the
host concatenates the per-core [1, 512] outputs.

Math notes (all exact reformulations, validated vs reference to ~1e-6):
  * softmax(where(mask, s, -1e9)) == exp(s)*mask / sum(exp(s)*mask)
    because |s| < 1 here (no max-subtraction needed) and exp(-1e9) == 0.
  * LayerNorm mean-centering is folded into the producing weight matrix
    (W - colmean(W)), so on-device LNs are variance-only.
  * LN2's affine (gain/bias) is folded into wk/wv and downstream biases.
  * V bias and the ones-normalization are folded into bo via softmax
    rows summing to 1 (denominator handled explicitly).
  * rsqrt(v+eps) = exp(-0.5*ln(v+eps)) keeps ACT in one table set (exp).
"""

import sys

if "/opt/trn_rl_repo" not in sys.path:
    sys.path.insert(0, "/opt/trn_rl_repo")

import numpy as np
import ml_dtypes

B, S, G, P, E, D, H, L = 4, 4096, 1024, 20, 3, 128, 4, 2
DK = D // H          # 32
GC = G // 2          # 512 genes per core
NCORES = 8
EPS = 1e-5
SCALE = 1.0 / float(np.sqrt(np.float32(DK)))
NCHUNK = S // 512    # 8
NT = S // 128        # 32 s-tiles

_CACHE = {}

BF16 = ml_dtypes.bfloat16
# Attention-path precision: True = bf16 attn weights/mask/V (fast DVE),
# False = all-f32 (accuracy reference).
ATT_BF16 = False
import os as _os
DEBUG_DUMPS = bool(_os.environ.get("KERNEL_DEBUG_DUMPS"))


def _build_program():
    import concourse.bacc as bacc
    import concourse.tile as tile
    from concourse import mybir

    f32 = mybir.dt.float32
    bf16 = mybir.dt.bfloat16
    at_dt = bf16 if ATT_BF16 else f32
    AF = mybir.ActivationFunctionType
    OP = mybir.AluOpType

    nc = bacc.Bacc("TRN2", debug=False, target_bir_lowering=False,
                   num_devices=NCORES)

    def din(name, shape, dt=f32):
        return nc.dram_tensor(name, list(shape), dt, kind="ExternalInput").ap()

    # ---- per-core data (differs across cores) ----
    d_xT = din("xT", [21, S])                    # [geno; pcs-rows] transposed
    d_maskT = din("maskT", [S, GC], bf16)        # cis mask^T slice, 0/1
    d_geneT = din("geneT", [D, GC])              # gene_emb^T slice
    d_film_a = din("film_a", [D, 1])             # 1 + tanh(env@gamma_w+gb)
    d_film_b = din("film_b", [D, 1])             # env@beta_w + bb
    d_bqdq = [din(f"bqdq{l}", [D, 1]) for l in range(L)]   # (bq+dq)*SCALE
    d_bkdk = [din(f"bkdk{l}", [D, 1]) for l in range(L)]   # bk+dk+ln2b@wk
    d_bo2c = [din(f"bo2c{l}", [D, 1]) for l in range(L)]   # centered bo''

    # ---- shared weights ----
    d_w1 = din("w1c", [21, 64])
    d_b1 = din("b1c", [64, 1])
    d_ln1g = din("ln1g", [64, 1])
    d_ln1b = din("ln1b", [64, 1])
    d_w2 = din("w2c", [64, D])
    d_b2 = din("b2c", [D, 1])
    d_wq = [din(f"wq{l}", [D, D]) for l in range(L)]
    d_wk = [din(f"wkg{l}", [D, D]) for l in range(L)]
    d_wv = [din(f"wvg{l}", [D, D]) for l in range(L)]
    d_wo = [din(f"woc{l}", [D, D], at_dt) for l in range(L)]
    d_lng = [din(f"lng{l}", [1, D]) for l in range(L)]
    d_lnb = [din(f"lnb{l}", [D, 1]) for l in range(L)]
    d_hw1 = din("hw1c", [D, 64])
    d_hb1 = din("hb1c", [64, 1])
    d_hlng = din("hlng", [1, 64])
    d_hlnb = din("hlnb", [64, 1])
    d_hw2 = din("hw2", [64, 1])
    d_hb2 = din("hb2", [1, 1])

    d_out = nc.dram_tensor("out", [1, GC], f32, kind="ExternalOutput").ap()
    dbg = {}
    if DEBUG_DUMPS:
        for nm, shp in [("dbg_h", [64, S]), ("dbg_sf", [D, S]),
                        ("dbg_hid1", [D, GC]), ("dbg_hid2", [D, GC]),
                        ("dbg_xln0", [D, GC]), ("dbg_rden0", [128, GC]),
                        ("dbg_kt0", [D, 512]), ("dbg_qt0", [D, GC])]:
            dbg[nm] = nc.dram_tensor(nm, shp, f32, kind="ExternalOutput").ap()

    with tile.TileContext(nc) as tc:
        with (
            tc.tile_pool(name="pw", bufs=1) as pw,          # persistent sbuf
            tc.tile_pool(name="pe", bufs=1) as pe,          # working sbuf
            tc.tile_pool(name="pa", bufs=3 if ATT_BF16 else 2) as pa,
            tc.tile_pool(name="pp", bufs=1, space="PSUM") as pp,
        ):
            dma = nc.sync.dma_start

            # ---------------- constants ----------------
            ones_col = pw.tile([128, 128], f32, tag="ones_col")
            nc.vector.memset(ones_col, 1.0)
            o64r = pw.tile([64, 32], f32, tag="o64r")       # 1/64 stats lhsT
            nc.vector.memset(o64r, 1.0 / 64.0)
            o128r = pw.tile([128, 32], f32, tag="o128r")    # 1/128 stats lhsT
            nc.vector.memset(o128r, 1.0 / 128.0)
            ones_bf = pw.tile([128, 1], at_dt, tag="ones_bf")
            nc.vector.memset(ones_bf, 1.0)
            eps128 = pw.tile([128, 1], f32, tag="eps128")
            nc.vector.memset(eps128, EPS)
            zeros128 = pw.tile([128, 128], f32, tag="zeros128")
            nc.vector.memset(zeros128, 0.0)

            # ---------------- load inputs ----------------
            x_sb = pw.tile([21, S], f32, tag="xT")
            dma(out=x_sb, in_=d_xT)
            w1_sb = pw.tile([21, 64], f32, tag="w1")
            dma(out=w1_sb, in_=d_w1)
            b1_sb = pw.tile([64, 1], f32, tag="b1")
            dma(out=b1_sb, in_=d_b1)
            ln1g_sb = pw.tile([64, 1], f32, tag="ln1g")
            dma(out=ln1g_sb, in_=d_ln1g)
            ln1b_sb = pw.tile([64, 1], f32, tag="ln1b")
            dma(out=ln1b_sb, in_=d_ln1b)
            w2_sb = pw.tile([64, D], f32, tag="w2")
            dma(out=w2_sb, in_=d_w2)
            b2_sb = pw.tile([D, 1], f32, tag="b2")
            dma(out=b2_sb, in_=d_b2)

            mask_sb = pw.tile([128, NT, GC], bf16, tag="mask")
            m_re = d_maskT.rearrange("(t p) g -> p t g", p=128)
            for q in range(4):
                dma(out=mask_sb[:, q * 8:(q + 1) * 8, :],
                    in_=m_re[:, q * 8:(q + 1) * 8, :])

            hid = [pw.tile([D, GC], f32, tag=f"hid{i}", name=f"hid{i}")
                   for i in range(L + 1)]
            dma(out=hid[0], in_=d_geneT)
            film_a_sb = pw.tile([D, 1], f32, tag="film_a")
            dma(out=film_a_sb, in_=d_film_a)
            film_b_sb = pw.tile([D, 1], f32, tag="film_b")
            dma(out=film_b_sb, in_=d_film_b)

            wq_sb, wk_sb, wv_sb, wo_sb = [], [], [], []
            bqdq_sb, bkdk_sb, bo2c_sb, lng_sb, lnb_sb = [], [], [], [], []
            for l in range(L):
                t_ = pw.tile([D, D], f32, tag=f"wq{l}")
                dma(out=t_, in_=d_wq[l]); wq_sb.append(t_)
                t_ = pw.tile([D, D], f32, tag=f"wk{l}")
                dma(out=t_, in_=d_wk[l]); wk_sb.append(t_)
                t_ = pw.tile([D, D], f32, tag=f"wv{l}")
                dma(out=t_, in_=d_wv[l]); wv_sb.append(t_)
                t_ = pw.tile([D, D], at_dt, tag=f"wo{l}")
                dma(out=t_, in_=d_wo[l]); wo_sb.append(t_)
                t_ = pw.tile([D, 1], f32, tag=f"bqdq{l}")
                dma(out=t_, in_=d_bqdq[l]); bqdq_sb.append(t_)
                t_ = pw.tile([D, 1], f32, tag=f"bkdk{l}")
                dma(out=t_, in_=d_bkdk[l]); bkdk_sb.append(t_)
                t_ = pw.tile([D, 1], f32, tag=f"bo2c{l}")
                dma(out=t_, in_=d_bo2c[l]); bo2c_sb.append(t_)
                t_ = pw.tile([1, D], f32, tag=f"lng{l}")
                dma(out=t_, in_=d_lng[l]); lng_sb.append(t_)
                t_ = pw.tile([D, 1], f32, tag=f"lnb{l}")
                dma(out=t_, in_=d_lnb[l]); lnb_sb.append(t_)
            hw1_sb = pw.tile([D, 64], f32, tag="hw1")
            dma(out=hw1_sb, in_=d_hw1)
            hb1_sb = pw.tile([64, 1], f32, tag="hb1")
            dma(out=hb1_sb, in_=d_hb1)
            hlng_sb = pw.tile([1, 64], f32, tag="hlng")
            dma(out=hlng_sb, in_=d_hlng)
            hlnb_sb = pw.tile([64, 1], f32, tag="hlnb")
            dma(out=hlnb_sb, in_=d_hlnb)
            hw2_sb = pw.tile([64, 1], f32, tag="hw2")
            dma(out=hw2_sb, in_=d_hw2)
            hb2_sb = pw.tile([1, 1], f32, tag="hb2")
            dma(out=hb2_sb, in_=d_hb2)

            # =============== SNP encoder (feature-major) ===============
            # LN1: h = relu(((x@w1c + b1c) * r1) * g1 + b1_ln)
            h_sb = pw.tile([64, S], f32, tag="h1")
            psV1 = pp.tile([128, 1024], f32, tag="ps_b", bufs=1,
                           name="psV1")
            for c in range(NCHUNK):
                sl = slice(c * 512, (c + 1) * 512)
                rg = 32 * (c % 4)
                psA = pp.tile([64, 512], f32, tag="ps_a", bufs=2, name="psA")
                nc.tensor.matmul(psA, w1_sb, x_sb[:, sl], start=True, stop=True)
                sqA = pe.tile([64, 512], f32, tag="sqA", bufs=3)
                nc.scalar.activation(sqA, psA, AF.Square, bias=b1_sb)
                nc.tensor.matmul(
                    psV1[rg:rg + 32, (c // 4) * 512:(c // 4 + 1) * 512],
                    o64r, sqA, start=True, stop=True, tile_position=(0, rg))
                nc.vector.tensor_scalar(h_sb[:, sl], psA, b1_sb, None, OP.add)
            rex1 = [None, None]
            for g_ in range(2):
                lnv = pe.tile([128, 512], f32, tag="lnv", bufs=2)
                nc.scalar.activation(lnv, psV1[:, g_ * 512:(g_ + 1) * 512],
                                     AF.Ln, bias=eps128)
                rex1[g_] = pe.tile([128, 512], f32, tag=f"rex1{g_}", bufs=1,
                                   name=f"rex1{g_}")
                nc.scalar.activation(rex1[g_], lnv, AF.Exp, scale=-0.5)
            for c in range(NCHUNK):
                sl = slice(c * 512, (c + 1) * 512)
                rg = 32 * (c % 4)
                psR = pp.tile([64, 512], f32, tag="ps_a", bufs=2, name="psR1")
                nc.tensor.matmul(psR, ones_col[rg:rg + 1, 0:64],
                                 rex1[c // 4][rg:rg + 1, :],
                                 start=True, stop=True, tile_position=(rg, 0))
                nc.vector.tensor_mul(h_sb[:, sl], h_sb[:, sl], psR)
                nc.scalar.activation(h_sb[:, sl], h_sb[:, sl], AF.Relu,
                                     bias=ln1b_sb, scale=ln1g_sb)

            # LN2 (affine folded into wk/wv): sf = (h@w2c + b2c) * r2
            sf_sb = pw.tile([D, S], f32, tag="snpf")
            psV2 = pp.tile([128, 1024], f32, tag="ps_b", bufs=1,
                           name="psV2")
            for c in range(NCHUNK):
                sl = slice(c * 512, (c + 1) * 512)
                rg = 32 * (c % 4)
                psB = pp.tile([128, 512], f32, tag="ps_a", bufs=2, name="psB")
                nc.tensor.matmul(psB, w2_sb, h_sb[:, sl], start=True, stop=True)
                sqB = pe.tile([128, 512], f32, tag="sqB", bufs=3)
                nc.scalar.activation(sqB, psB, AF.Square, bias=b2_sb)
                nc.tensor.matmul(
                    psV2[rg:rg + 32, (c // 4) * 512:(c // 4 + 1) * 512],
                    o128r, sqB, start=True, stop=True, tile_position=(0, rg))
                nc.vector.tensor_scalar(sf_sb[:, sl], psB, b2_sb, None, OP.add)
            rex2 = [None, None]
            for g_ in range(2):
                lnv = pe.tile([128, 512], f32, tag="lnv", bufs=2)
                nc.scalar.activation(lnv, psV2[:, g_ * 512:(g_ + 1) * 512],
                                     AF.Ln, bias=eps128)
                rex2[g_] = pe.tile([128, 512], f32, tag=f"rex2{g_}", bufs=1,
                                   name=f"rex2{g_}")
                nc.scalar.activation(rex2[g_], lnv, AF.Exp, scale=-0.5)
            for c in range(NCHUNK):
                sl = slice(c * 512, (c + 1) * 512)
                rg = 32 * (c % 4)
                psR = pp.tile([128, 512], f32, tag="ps_a", bufs=2, name="psR2e")
                nc.tensor.matmul(psR, ones_col[rg:rg + 1, 0:128],
                                 rex2[c // 4][rg:rg + 1, :],
                                 start=True, stop=True, tile_position=(rg, 0))
                nc.vector.tensor_mul(sf_sb[:, sl], sf_sb[:, sl], psR)

            if DEBUG_DUMPS:
                dma(out=dbg["dbg_h"], in_=h_sb)
                dma(out=dbg["dbg_sf"], in_=sf_sb)

            # =============== transformer layers ===============
            for l in range(L):
                # Q^T [D, GC] bf16, pre-scaled by 1/sqrt(dk)
                psQ = pp.tile([D, GC], f32, tag="ps_a", bufs=2, name="psQ")
                nc.tensor.matmul(psQ, wq_sb[l], hid[l], start=True, stop=True)
                QT = pe.tile([D, GC], at_dt, tag="QT")
                nc.scalar.activation(QT, psQ, AF.Identity,
                                     bias=bqdq_sb[l], scale=SCALE)
                # K^T [D, S] bf16
                KT = pe.tile([D, S], at_dt, tag="KT")
                for c in range(NCHUNK):
                    sl = slice(c * 512, (c + 1) * 512)
                    psK = pp.tile([D, 512], f32, tag="ps_a", bufs=2, name="psK")
                    nc.tensor.matmul(psK, wk_sb[l], sf_sb[:, sl],
                                     start=True, stop=True)
                    nc.vector.tensor_scalar(KT[:, sl], psK, bkdk_sb[l],
                                            None, OP.add)
                if DEBUG_DUMPS and l == 0:
                    qtf = pe.tile([D, GC], f32, tag="qtf")
                    nc.vector.tensor_copy(qtf, QT)
                    dma(out=dbg["dbg_qt0"], in_=qtf)
                # V [s-local, (t, dv)] bf16
                Vb = pe.tile([128, S], at_dt, tag="Vb")
                for q in range(NCHUNK):
                    psv = pp.tile([128, 512], f32, tag="ps_a", bufs=2, name="psv")
                    for k in range(4):
                        t = 4 * q + k
                        nc.tensor.matmul(psv[:, k * 128:(k + 1) * 128],
                                         sf_sb[:, t * 128:(t + 1) * 128],
                                         wv_sb[l], start=True, stop=True)
                    nc.vector.tensor_copy(Vb[:, q * 512:(q + 1) * 512], psv)

                # attention: scores^T -> exp -> mask -> attn@V (+denoms)
                # psOD bank h: attnV out for head h at partitions [32h,32h+32)
                # and its denominator row at partition 32*((h+1)%4).
                psOD = pp.tile([128, 4 * 512], f32, tag="ps_b", bufs=1, name="psOD")
                for h in range(H):
                    nc.tensor.matmul(psOD[:, h * 512:(h + 1) * 512],
                                     zeros128, sf_sb[:, 0:512],
                                     start=True, stop=False,
                                     skip_group_check=True)
                for t in range(NT):
                    at = pa.tile([128, 4 * 512], at_dt, tag="attn")
                    for hp in range(2):
                        psS = pp.tile([128, 2 * 512], f32, tag="ps_a",
                                      bufs=2, name="psS")
                        for hh in range(2):
                            h = 2 * hp + hh
                            nc.tensor.matmul(
                                psS[:, hh * 512:(hh + 1) * 512],
                                KT[32 * h:32 * h + 32, t * 128:(t + 1) * 128],
                                QT[32 * h:32 * h + 32, :],
                                start=True, stop=True,
                                tile_position=(32 * h, 0))
                        nc.scalar.activation(
                            at[:, hp * 1024:(hp + 1) * 1024], psS, AF.Exp)
                    for h in range(H):
                        nc.vector.tensor_mul(at[:, h * 512:(h + 1) * 512],
                                             at[:, h * 512:(h + 1) * 512],
                                             mask_sb[:, t, :])
                    for h in range(H):
                        nc.tensor.matmul(
                            psOD[32 * h:32 * h + 32, h * 512:(h + 1) * 512],
                            Vb[:, t * 128 + 32 * h:t * 128 + 32 * h + 32],
                            at[:, h * 512:(h + 1) * 512],
                            start=False, stop=False,
                            tile_position=(0, 32 * h),
                            skip_group_check=True)
                    for h in range(H):
                        jh = 32 * ((h + 1) % 4)
                        nc.tensor.matmul(
                            psOD[jh:jh + 1, h * 512:(h + 1) * 512],
                            ones_bf,
                            at[:, h * 512:(h + 1) * 512],
                            start=False, stop=(t == NT - 1),
                            tile_position=(0, jh),
                            skip_group_check=True)

                # normalize by denominators, project, LN, residual
                rden = pe.tile([128, GC], f32, tag="rden")
                if DEBUG_DUMPS:
                    nc.vector.memset(rden, 0.0)
                for h in range(H):
                    jh = 32 * ((h + 1) % 4)
                    nc.vector.reciprocal(rden[jh:jh + 1, :],
                                         psOD[jh:jh + 1, h * 512:(h + 1) * 512])
                psRb = pp.tile([128, GC], f32, tag="ps_a", bufs=2, name="psRb")
                for h in range(H):
                    jh = 32 * ((h + 1) % 4)
                    nc.tensor.matmul(psRb[32 * h:32 * h + 32, :],
                                     ones_col[jh:jh + 1, 0:32],
                                     rden[jh:jh + 1, :],
                                     start=True, stop=True,
                                     tile_position=(jh, 32 * h))
                rbc = pe.tile([128, GC], f32, tag="rbc_sb")
                nc.scalar.activation(rbc, psRb, AF.Copy)
                outn = pe.tile([128, GC], at_dt, tag="outn")
                for h in range(H):
                    hp = slice(32 * h, 32 * h + 32)
                    nc.vector.tensor_mul(outn[hp, :],
                                         psOD[hp, h * 512:(h + 1) * 512],
                                         rbc[hp, :])

                psP = pp.tile([D, GC], f32, tag="ps_a", bufs=2, name="psP")
                nc.tensor.matmul(psP, wo_sb[l], outn, start=True, stop=True)
                xsb = pe.tile([D, GC], f32, tag="xln")
                nc.scalar.activation(xsb, psP, AF.Identity, bias=bo2c_sb[l])
                sq = pe.tile([D, GC], f32, tag="sqln")
                nc.scalar.activation(sq, psP, AF.Square, bias=bo2c_sb[l])
                psVr = pp.tile([32, GC], f32, tag="ps_a", bufs=2, name="psVr")
                nc.tensor.matmul(psVr, o128r, sq, start=True, stop=True)
                lnv2 = pe.tile([32, GC], f32, tag="lnv2")
                nc.scalar.activation(lnv2, psVr, AF.Ln, bias=eps128[0:32, :])
                rex = pe.tile([32, GC], f32, tag="rexln")
                nc.scalar.activation(rex, lnv2, AF.Exp, scale=-0.5)
                psR2 = pp.tile([D, GC], f32, tag="ps_a", bufs=2, name="psR2")
                nc.tensor.matmul(psR2, lng_sb[l], rex[0:1, :],
                                 start=True, stop=True)
                y = pe.tile([D, GC], f32, tag="yln")
                nc.vector.tensor_mul(y, xsb, psR2)
                nc.vector.tensor_scalar(y, y, lnb_sb[l], None, OP.add)
                nc.vector.tensor_add(hid[l + 1], hid[l], y)
                if DEBUG_DUMPS:
                    dma(out=dbg[f"dbg_hid{l + 1}"], in_=hid[l + 1])
                    if l == 0:
                        dma(out=dbg["dbg_rden0"], in_=rden)
                        dma(out=dbg["dbg_xln0"], in_=xsb)
                        ktf = pe.tile([D, 512], f32, tag="ktf")
                        nc.vector.tensor_copy(ktf, KT[:, 0:512])
                        dma(out=dbg["dbg_kt0"], in_=ktf)

            # =============== FiLM + gene head ===============
            rep = pe.tile([D, GC], f32, tag="rep")
            nc.vector.tensor_scalar(rep, hid[L], film_a_sb, film_b_sb,
                                    OP.mult, OP.add)
            psH = pp.tile([64, GC], f32, tag="ps_a", bufs=2, name="psH")
            nc.tensor.matmul(psH, hw1_sb, rep, start=True, stop=True)
            hs = pe.tile([64, GC], f32, tag="hs")
            nc.scalar.activation(hs, psH, AF.Identity, bias=hb1_sb)
            sqh = pe.tile([64, GC], f32, tag="sqh")
            nc.scalar.activation(sqh, psH, AF.Square, bias=hb1_sb)
            psVh = pp.tile([32, GC], f32, tag="ps_a", bufs=2, name="psVh")
            nc.tensor.matmul(psVh, o64r, sqh, start=True, stop=True)
            lnvh = pe.tile([32, GC], f32, tag="lnvh")
            nc.scalar.activation(lnvh, psVh, AF.Ln, bias=eps128[0:32, :])
            rexh = pe.tile([32, GC], f32, tag="rexh")
            nc.scalar.activation(rexh, lnvh, AF.Exp, scale=-0.5)
            psRh = pp.tile([64, GC], f32, tag="ps_a", bufs=2, name="psRh")
            nc.tensor.matmul(psRh, hlng_sb, rexh[0:1, :], start=True, stop=True)
            yh = pe.tile([64, GC], f32, tag="yh")
            nc.vector.tensor_mul(yh, hs, psRh)
            hh = pe.tile([64, GC], f32, tag="hh")
            nc.scalar.activation(hh, yh, AF.Relu, bias=hlnb_sb)
            psp = pp.tile([1, GC], f32, tag="ps_a", bufs=2, name="psp")
            nc.tensor.matmul(psp, hw2_sb, hh, start=True, stop=True)
            pr = pe.tile([1, GC], f32, tag="pr")
            nc.scalar.activation(pr, psp, AF.Identity, bias=hb2_sb)
            dma(out=d_out, in_=pr)

    nc.compile()
    return nc


def _fold_inputs(inp):
    """Host-side exact folding of small per-batch vectors and LN
    centering into weights. Returns (shared, per_core) input dicts."""
    f = np.float32
    i = {k: np.asarray(v) for k, v in inp.items()}

    def colcenter(w):
        return (w - w.mean(axis=1, keepdims=True)).astype(f)

    def vcenter(b):
        return (b - b.mean()).astype(f)

    shared = {
        "w1c": colcenter(i["snp_w1"]),
        "b1c": vcenter(i["snp_b1"])[:, None],
        "ln1g": i["snp_ln1_g"].astype(f)[:, None],
        "ln1b": i["snp_ln1_b"].astype(f)[:, None],
        "w2c": colcenter(i["snp_w2"]),
        "b2c": vcenter(i["snp_b2"])[:, None],
        "hw1c": colcenter(i["head_w1"]),
        "hb1c": vcenter(i["head_b1"])[:, None],
        "hlng": i["head_ln_g"].astype(f)[None, :],
        "hlnb": i["head_ln_b"].astype(f)[:, None],
        "hw2": i["head_w2"].astype(f),
        "hb2": i["head_b2"].astype(f)[None, :],
    }
    g2 = i["snp_ln2_g"]
    b2 = i["snp_ln2_b"]
    for l in range(L):
        shared[f"wq{l}"] = i["wq"][l].astype(f)
        shared[f"wkg{l}"] = (g2[:, None] * i["wk"][l]).astype(f)
        shared[f"wvg{l}"] = (g2[:, None] * i["wv"][l]).astype(f)
        shared[f"woc{l}"] = colcenter(i["wo"][l]).astype(BF16 if ATT_BF16 else np.float32)
        shared[f"lng{l}"] = i["ln_g"][l].astype(f)[None, :]
        shared[f"lnb{l}"] = i["ln_b"][l].astype(f)[:, None]

    env_all = i["env_emb"][i["env_indices"].astype(np.int64)]  # [B, D]
    per_core = []
    for c in range(NCORES):
        b, j = c // 2, c % 2
        env = env_all[b]
        dq = env @ i["wq_env"]
        dk = env @ i["wk_env"]
        m = {
            "xT": np.concatenate(
                [i["genotypes"][b][None, :],
                 np.repeat(i["pcs"][b][:, None], S, axis=1)], 0).astype(f),
            "maskT": np.ascontiguousarray(
                i["cis_mask"].T[:, j * GC:(j + 1) * GC]).astype(BF16),
            "geneT": np.ascontiguousarray(
                i["gene_emb"][j * GC:(j + 1) * GC].T).astype(f),
            "film_a": (1.0 + np.tanh(env @ i["gamma_w"] + i["gamma_b"])
                       ).astype(f)[:, None],
            "film_b": (env @ i["beta_w"] + i["beta_b"]).astype(f)[:, None],
        }
        for l in range(L):
            cv = i["bv"][l] + b2 @ i["wv"][l]
            bo2 = i["bo"][l] + cv @ i["wo"][l]
            m[f"bqdq{l}"] = ((i["bq"][l] + dq) * SCALE).astype(f)[:, None]
            m[f"bkdk{l}"] = (i["bk"][l] + dk + b2 @ i["wk"][l]
                             ).astype(f)[:, None]
            m[f"bo2c{l}"] = vcenter(bo2)[:, None]
        per_core.append(m)
    return shared, per_core


def _get_program():
    if "nc" not in _CACHE:
        _CACHE["nc"] = _build_program()
    return _CACHE["nc"]


def run_cores(inp, trace=False):
    """Build in_maps, run on the 8 NeuronCores, return (preds, results)."""
    from concourse import bass_utils
    nc = _get_program()
    shared, per_core = _fold_inputs(inp)
    in_maps = [{**shared, **pc} for pc in per_core]
    res = bass_utils.run_bass_kernel_spmd(
        nc, in_maps, core_ids=list(range(NCORES)), trace=trace)
    preds = np.zeros((B, G), np.float32)
    for c in range(NCORES):
        b, j = c // 2, c % 2
        preds[b, j * GC:(j + 1) * GC] = res.results[c]["out"][0]
    return preds, res


def kernel(**inputs) -> np.ndarray:
    preds, _ = run_cores(inputs, trace=False)
    return preds


# revision 17
# speedup vs baseline: 1.6493x; 1.6493x over previous
"""Trainium2 Bass kernel for nn_EnvironmentConditionalTransformer.

Contract: kernel(**inputs) takes the FULL unsharded inputs (numpy arrays,
keys as in setup_inputs()) and returns the FULL [B, G] float32 output.

Sharding: 8 cores = batch(4) x gene-halves(2). Each core runs the SNP
encoder for its batch over all S=4096 SNPs, projects K/V, and computes
masked attention + FiLM + head for its 512 genes. Zero collectives; the
host concatenates the per-core [1, 512] outputs.

Math notes (all exact reformulations, validated vs reference to ~1e-6):
  * softmax(where(mask, s, -1e9)) == exp(s)*mask / sum(exp(s)*mask)
    because |s| < 1 here (no max-subtraction needed) and exp(-1e9) == 0.
  * LayerNorm mean-centering is folded into the producing weight matrix
    (W - colmean(W)), so on-device LNs are variance-only.
  * LN2's affine (gain/bias) is folded into wk/wv and downstream biases.
  * V bias and the ones-normalization are folded into bo via softmax
    rows summing to 1 (denominator handled explicitly).
  * rsqrt(v+eps) = exp(-0.5*ln(v+eps)) keeps ACT in one table set (exp).
"""

import sys

if "/opt/trn_rl_repo" not in sys.path:
    sys.path.insert(0, "/opt/trn_rl_repo")

import numpy as np
import ml_dtypes

B, S, G, P, E, D, H, L = 4, 4096, 1024, 20, 3, 128, 4, 2
DK = D // H          # 32
GC = G // 2          # 512 genes per core
NCORES = 8
EPS = 1e-5
SCALE = 1.0 / float(np.sqrt(np.float32(DK)))
NCHUNK = S // 512    # 8
NT = S // 128        # 32 s-tiles

_CACHE = {}

BF16 = ml_dtypes.bfloat16
# Attention-path precision: True = bf16 attn weights/mask/V (fast DVE),
# False = all-f32 (accuracy reference).
ATT_BF16 = True
import os as _os
DEBUG_DUMPS = bool(_os.environ.get("KERNEL_DEBUG_DUMPS"))


def _build_program():
    import concourse.bacc as bacc
    import concourse.tile as tile
    from concourse import mybir

    f32 = mybir.dt.float32
    bf16 = mybir.dt.bfloat16
    at_dt = bf16 if ATT_BF16 else f32
    AF = mybir.ActivationFunctionType
    OP = mybir.AluOpType

    nc = bacc.Bacc("TRN2", debug=False, target_bir_lowering=False,
                   num_devices=NCORES)

    def din(name, shape, dt=f32):
        return nc.dram_tensor(name, list(shape), dt, kind="ExternalInput").ap()

    # ---- per-core data (differs across cores) ----
    d_xT = din("xT", [21, S])                    # [geno; pcs-rows] transposed
    d_maskT = din("maskT", [S, GC], bf16)        # cis mask^T slice, 0/1
    d_geneT = din("geneT", [D, GC])              # gene_emb^T slice
    d_film_a = din("film_a", [D, 1])             # 1 + tanh(env@gamma_w+gb)
    d_film_b = din("film_b", [D, 1])             # env@beta_w + bb
    d_bqdq = [din(f"bqdq{l}", [D, 1]) for l in range(L)]   # (bq+dq)*SCALE
    d_bkdk = [din(f"bkdk{l}", [D, 1]) for l in range(L)]   # bk+dk+ln2b@wk
    d_bo2c = [din(f"bo2c{l}", [D, 1]) for l in range(L)]   # centered bo''

    # ---- shared weights ----
    d_w1 = din("w1c", [21, 64])
    d_b1 = din("b1c", [64, 1])
    d_ln1g = din("ln1g", [64, 1])
    d_ln1b = din("ln1b", [64, 1])
    d_w2 = din("w2c", [64, D])
    d_b2 = din("b2c", [D, 1])
    d_wq = [din(f"wq{l}", [D, D]) for l in range(L)]
    d_wk = [din(f"wkg{l}", [D, D]) for l in range(L)]
    d_wv = [din(f"wvg{l}", [D, D]) for l in range(L)]
    d_wo = [din(f"woc{l}", [D, D], at_dt) for l in range(L)]
    d_lng = [din(f"lng{l}", [1, D]) for l in range(L)]
    d_lnb = [din(f"lnb{l}", [D, 1]) for l in range(L)]
    d_hw1 = din("hw1c", [D, 64])
    d_hb1 = din("hb1c", [64, 1])
    d_hlng = din("hlng", [1, 64])
    d_hlnb = din("hlnb", [64, 1])
    d_hw2 = din("hw2", [64, 1])
    d_hb2 = din("hb2", [1, 1])

    d_out = nc.dram_tensor("out", [1, GC], f32, kind="ExternalOutput").ap()
    dbg = {}
    if DEBUG_DUMPS:
        for nm, shp in [("dbg_h", [64, S]), ("dbg_sf", [D, S]),
                        ("dbg_hid1", [D, GC]), ("dbg_hid2", [D, GC]),
                        ("dbg_xln0", [D, GC]), ("dbg_rden0", [128, GC]),
                        ("dbg_kt0", [D, 512]), ("dbg_qt0", [D, GC])]:
            dbg[nm] = nc.dram_tensor(nm, shp, f32, kind="ExternalOutput").ap()

    with tile.TileContext(nc) as tc:
        with (
            tc.tile_pool(name="pw", bufs=1) as pw,          # persistent sbuf
            tc.tile_pool(name="pe", bufs=1) as pe,          # working sbuf
            tc.tile_pool(name="pa", bufs=3 if ATT_BF16 else 2) as pa,
            tc.tile_pool(name="pp", bufs=1, space="PSUM") as pp,
        ):
            dma = nc.sync.dma_start

            # ---------------- constants ----------------
            ones_col = pw.tile([128, 128], f32, tag="ones_col")
            nc.vector.memset(ones_col, 1.0)
            o64r = pw.tile([64, 32], f32, tag="o64r")       # 1/64 stats lhsT
            nc.vector.memset(o64r, 1.0 / 64.0)
            o128r = pw.tile([128, 32], f32, tag="o128r")    # 1/128 stats lhsT
            nc.vector.memset(o128r, 1.0 / 128.0)
            ones_bf = pw.tile([128, 1], at_dt, tag="ones_bf")
            nc.vector.memset(ones_bf, 1.0)
            eps128 = pw.tile([128, 1], f32, tag="eps128")
            nc.vector.memset(eps128, EPS)
            zeros128 = pw.tile([128, 128], f32, tag="zeros128")
            nc.vector.memset(zeros128, 0.0)

            # ---------------- load inputs ----------------
            x_sb = pw.tile([21, S], f32, tag="xT")
            dma(out=x_sb, in_=d_xT)
            w1_sb = pw.tile([21, 64], f32, tag="w1")
            dma(out=w1_sb, in_=d_w1)
            b1_sb = pw.tile([64, 1], f32, tag="b1")
            dma(out=b1_sb, in_=d_b1)
            ln1g_sb = pw.tile([64, 1], f32, tag="ln1g")
            dma(out=ln1g_sb, in_=d_ln1g)
            ln1b_sb = pw.tile([64, 1], f32, tag="ln1b")
            dma(out=ln1b_sb, in_=d_ln1b)
            w2_sb = pw.tile([64, D], f32, tag="w2")
            dma(out=w2_sb, in_=d_w2)
            b2_sb = pw.tile([D, 1], f32, tag="b2")
            dma(out=b2_sb, in_=d_b2)

            mask_sb = pw.tile([128, NT, GC], bf16, tag="mask")
            m_re = d_maskT.rearrange("(t p) g -> p t g", p=128)
            for q in range(4):
                dma(out=mask_sb[:, q * 8:(q + 1) * 8, :],
                    in_=m_re[:, q * 8:(q + 1) * 8, :])

            hid = [pw.tile([D, GC], f32, tag=f"hid{i}", name=f"hid{i}")
                   for i in range(L + 1)]
            dma(out=hid[0], in_=d_geneT)
            film_a_sb = pw.tile([D, 1], f32, tag="film_a")
            dma(out=film_a_sb, in_=d_film_a)
            film_b_sb = pw.tile([D, 1], f32, tag="film_b")
            dma(out=film_b_sb, in_=d_film_b)

            wq_sb, wk_sb, wv_sb, wo_sb = [], [], [], []
            bqdq_sb, bkdk_sb, bo2c_sb, lng_sb, lnb_sb = [], [], [], [], []
            for l in range(L):
                t_ = pw.tile([D, D], f32, tag=f"wq{l}")
                dma(out=t_, in_=d_wq[l]); wq_sb.append(t_)
                t_ = pw.tile([D, D], f32, tag=f"wk{l}")
                dma(out=t_, in_=d_wk[l]); wk_sb.append(t_)
                t_ = pw.tile([D, D], f32, tag=f"wv{l}")
                dma(out=t_, in_=d_wv[l]); wv_sb.append(t_)
                t_ = pw.tile([D, D], at_dt, tag=f"wo{l}")
                dma(out=t_, in_=d_wo[l]); wo_sb.append(t_)
                t_ = pw.tile([D, 1], f32, tag=f"bqdq{l}")
                dma(out=t_, in_=d_bqdq[l]); bqdq_sb.append(t_)
                t_ = pw.tile([D, 1], f32, tag=f"bkdk{l}")
                dma(out=t_, in_=d_bkdk[l]); bkdk_sb.append(t_)
                t_ = pw.tile([D, 1], f32, tag=f"bo2c{l}")
                dma(out=t_, in_=d_bo2c[l]); bo2c_sb.append(t_)
                t_ = pw.tile([1, D], f32, tag=f"lng{l}")
                dma(out=t_, in_=d_lng[l]); lng_sb.append(t_)
                t_ = pw.tile([D, 1], f32, tag=f"lnb{l}")
                dma(out=t_, in_=d_lnb[l]); lnb_sb.append(t_)
            hw1_sb = pw.tile([D, 64], f32, tag="hw1")
            dma(out=hw1_sb, in_=d_hw1)
            hb1_sb = pw.tile([64, 1], f32, tag="hb1")
            dma(out=hb1_sb, in_=d_hb1)
            hlng_sb = pw.tile([1, 64], f32, tag="hlng")
            dma(out=hlng_sb, in_=d_hlng)
            hlnb_sb = pw.tile([64, 1], f32, tag="hlnb")
            dma(out=hlnb_sb, in_=d_hlnb)
            hw2_sb = pw.tile([64, 1], f32, tag="hw2")
            dma(out=hw2_sb, in_=d_hw2)
            hb2_sb = pw.tile([1, 1], f32, tag="hb2")
            dma(out=hb2_sb, in_=d_hb2)

            # =============== SNP encoder (feature-major) ===============
            # LN1: h = relu(((x@w1c + b1c) * r1) * g1 + b1_ln)
            h_sb = pw.tile([64, S], f32, tag="h1")
            psV1 = pp.tile([128, 1024], f32, tag="ps_b", bufs=1,
                           name="psV1")
            for c in range(NCHUNK):
                sl = slice(c * 512, (c + 1) * 512)
                rg = 32 * (c % 4)
                psA = pp.tile([64, 512], f32, tag="ps_a", bufs=2, name="psA")
                nc.tensor.matmul(psA, w1_sb, x_sb[:, sl], start=True, stop=True)
                sqA = pe.tile([64, 512], f32, tag="sqA", bufs=3)
                nc.scalar.activation(sqA, psA, AF.Square, bias=b1_sb)
                nc.tensor.matmul(
                    psV1[rg:rg + 32, (c // 4) * 512:(c // 4 + 1) * 512],
                    o64r, sqA, start=True, stop=True, tile_position=(0, rg))
                nc.vector.tensor_scalar(h_sb[:, sl], psA, b1_sb, None, OP.add)
            rex1 = [None, None]
            for g_ in range(2):
                lnv = pe.tile([128, 512], f32, tag="lnv", bufs=2)
                nc.scalar.activation(lnv, psV1[:, g_ * 512:(g_ + 1) * 512],
                                     AF.Ln, bias=eps128)
                rex1[g_] = pe.tile([128, 512], f32, tag=f"rex1{g_}", bufs=1,
                                   name=f"rex1{g_}")
                nc.scalar.activation(rex1[g_], lnv, AF.Exp, scale=-0.5)
            for c in range(NCHUNK):
                sl = slice(c * 512, (c + 1) * 512)
                rg = 32 * (c % 4)
                psR = pp.tile([64, 512], f32, tag="ps_a", bufs=2, name="psR1")
                nc.tensor.matmul(psR, ones_col[rg:rg + 1, 0:64],
                                 rex1[c // 4][rg:rg + 1, :],
                                 start=True, stop=True, tile_position=(rg, 0))
                nc.vector.tensor_mul(h_sb[:, sl], h_sb[:, sl], psR)
                nc.scalar.activation(h_sb[:, sl], h_sb[:, sl], AF.Relu,
                                     bias=ln1b_sb, scale=ln1g_sb)

            # LN2 (affine folded into wk/wv): sf = (h@w2c + b2c) * r2
            sf_sb = pw.tile([D, S], f32, tag="snpf")
            psV2 = pp.tile([128, 1024], f32, tag="ps_b", bufs=1,
                           name="psV2")
            for c in range(NCHUNK):
                sl = slice(c * 512, (c + 1) * 512)
                rg = 32 * (c % 4)
                psB = pp.tile([128, 512], f32, tag="ps_a", bufs=2, name="psB")
                nc.tensor.matmul(psB, w2_sb, h_sb[:, sl], start=True, stop=True)
                sqB = pe.tile([128, 512], f32, tag="sqB", bufs=3)
                nc.scalar.activation(sqB, psB, AF.Square, bias=b2_sb)
                nc.tensor.matmul(
                    psV2[rg:rg + 32, (c // 4) * 512:(c // 4 + 1) * 512],
                    o128r, sqB, start=True, stop=True, tile_position=(0, rg))
                nc.vector.tensor_scalar(sf_sb[:, sl], psB, b2_sb, None, OP.add)
            rex2 = [None, None]
            for g_ in range(2):
                lnv = pe.tile([128, 512], f32, tag="lnv", bufs=2)
                nc.scalar.activation(lnv, psV2[:, g_ * 512:(g_ + 1) * 512],
                                     AF.Ln, bias=eps128)
                rex2[g_] = pe.tile([128, 512], f32, tag=f"rex2{g_}", bufs=1,
                                   name=f"rex2{g_}")
                nc.scalar.activation(rex2[g_], lnv, AF.Exp, scale=-0.5)
            for c in range(NCHUNK):
                sl = slice(c * 512, (c + 1) * 512)
                rg = 32 * (c % 4)
                psR = pp.tile([128, 512], f32, tag="ps_a", bufs=2, name="psR2e")
                nc.tensor.matmul(psR, ones_col[rg:rg + 1, 0:128],
                                 rex2[c // 4][rg:rg + 1, :],
                                 start=True, stop=True, tile_position=(rg, 0))
                nc.vector.tensor_mul(sf_sb[:, sl], sf_sb[:, sl], psR)

            if DEBUG_DUMPS:
                dma(out=dbg["dbg_h"], in_=h_sb)
                dma(out=dbg["dbg_sf"], in_=sf_sb)

            # =============== transformer layers ===============
            for l in range(L):
                # Q^T [D, GC] bf16, pre-scaled by 1/sqrt(dk)
                psQ = pp.tile([D, GC], f32, tag="ps_a", bufs=2, name="psQ")
                nc.tensor.matmul(psQ, wq_sb[l], hid[l], start=True, stop=True)
                QT = pe.tile([D, GC], at_dt, tag="QT")
                nc.scalar.activation(QT, psQ, AF.Identity,
                                     bias=bqdq_sb[l], scale=SCALE)
                # K^T [D, S] bf16
                KT = pe.tile([D, S], at_dt, tag="KT")
                for c in range(NCHUNK):
                    sl = slice(c * 512, (c + 1) * 512)
                    psK = pp.tile([D, 512], f32, tag="ps_a", bufs=2, name="psK")
                    nc.tensor.matmul(psK, wk_sb[l], sf_sb[:, sl],
                                     start=True, stop=True)
                    nc.vector.tensor_scalar(KT[:, sl], psK, bkdk_sb[l],
                                            None, OP.add)
                if DEBUG_DUMPS and l == 0:
                    qtf = pe.tile([D, GC], f32, tag="qtf")
                    nc.vector.tensor_copy(qtf, QT)
                    dma(out=dbg["dbg_qt0"], in_=qtf)
                # V [s-local, (t, dv)] bf16
                Vb = pe.tile([128, S], at_dt, tag="Vb")
                for q in range(NCHUNK):
                    psv = pp.tile([128, 512], f32, tag="ps_a", bufs=2, name="psv")
                    for k in range(4):
                        t = 4 * q + k
                        nc.tensor.matmul(psv[:, k * 128:(k + 1) * 128],
                                         sf_sb[:, t * 128:(t + 1) * 128],
                                         wv_sb[l], start=True, stop=True)
                    nc.vector.tensor_copy(Vb[:, q * 512:(q + 1) * 512], psv)

                # attention: scores^T -> exp -> mask -> attn@V (+denoms)
                # psOD bank h: attnV out for head h at partitions [32h,32h+32)
                # and its denominator row at partition 32*((h+1)%4).
                psOD = pp.tile([128, 4 * 512], f32, tag="ps_b", bufs=1, name="psOD")
                for h in range(H):
                    nc.tensor.matmul(psOD[:, h * 512:(h + 1) * 512],
                                     zeros128, sf_sb[:, 0:512],
                                     start=True, stop=False,
                                     skip_group_check=True)
                for t in range(NT):
                    at = pa.tile([128, 4 * 512], at_dt, tag="attn")
                    for hp in range(2):
                        psS = pp.tile([128, 2 * 512], f32, tag="ps_a",
                                      bufs=2, name="psS")
                        for hh in range(2):
                            h = 2 * hp + hh
                            nc.tensor.matmul(
                                psS[:, hh * 512:(hh + 1) * 512],
                                KT[32 * h:32 * h + 32, t * 128:(t + 1) * 128],
                                QT[32 * h:32 * h + 32, :],
                                start=True, stop=True,
                                tile_position=(32 * h, 0))
                        nc.scalar.activation(
                            at[:, hp * 1024:(hp + 1) * 1024], psS, AF.Exp)
                    for h in range(H):
                        nc.vector.tensor_mul(at[:, h * 512:(h + 1) * 512],
                                             at[:, h * 512:(h + 1) * 512],
                                             mask_sb[:, t, :])
                    for h in range(H):
                        nc.tensor.matmul(
                            psOD[32 * h:32 * h + 32, h * 512:(h + 1) * 512],
                            Vb[:, t * 128 + 32 * h:t * 128 + 32 * h + 32],
                            at[:, h * 512:(h + 1) * 512],
                            start=False, stop=False,
                            tile_position=(0, 32 * h),
                            skip_group_check=True)
                    for h in range(H):
                        jh = 32 * ((h + 1) % 4)
                        nc.tensor.matmul(
                            psOD[jh:jh + 1, h * 512:(h + 1) * 512],
                            ones_bf,
                            at[:, h * 512:(h + 1) * 512],
                            start=False, stop=(t == NT - 1),
                            tile_position=(0, jh),
                            skip_group_check=True)

                # normalize by denominators, project, LN, residual
                rden = pe.tile([128, GC], f32, tag="rden")
                if DEBUG_DUMPS:
                    nc.vector.memset(rden, 0.0)
                for h in range(H):
                    jh = 32 * ((h + 1) % 4)
                    nc.vector.reciprocal(rden[jh:jh + 1, :],
                                         psOD[jh:jh + 1, h * 512:(h + 1) * 512])
                psRb = pp.tile([128, GC], f32, tag="ps_a", bufs=2, name="psRb")
                for h in range(H):
                    jh = 32 * ((h + 1) % 4)
                    nc.tensor.matmul(psRb[32 * h:32 * h + 32, :],
                                     ones_col[jh:jh + 1, 0:32],
                                     rden[jh:jh + 1, :],
                                     start=True, stop=True,
                                     tile_position=(jh, 32 * h))
                rbc = pe.tile([128, GC], f32, tag="rbc_sb")
                nc.scalar.activation(rbc, psRb, AF.Copy)
                outn = pe.tile([128, GC], at_dt, tag="outn")
                for h in range(H):
                    hp = slice(32 * h, 32 * h + 32)
                    nc.vector.tensor_mul(outn[hp, :],
                                         psOD[hp, h * 512:(h + 1) * 512],
                                         rbc[hp, :])

                psP = pp.tile([D, GC], f32, tag="ps_a", bufs=2, name="psP")
                nc.tensor.matmul(psP, wo_sb[l], outn, start=True, stop=True)
                xsb = pe.tile([D, GC], f32, tag="xln")
                nc.scalar.activation(xsb, psP, AF.Identity, bias=bo2c_sb[l])
                sq = pe.tile([D, GC], f32, tag="sqln")
                nc.scalar.activation(sq, psP, AF.Square, bias=bo2c_sb[l])
                psVr = pp.tile([32, GC], f32, tag="ps_a", bufs=2, name="psVr")
                nc.tensor.matmul(psVr, o128r, sq, start=True, stop=True)
                lnv2 = pe.tile([32, GC], f32, tag="lnv2")
                nc.scalar.activation(lnv2, psVr, AF.Ln, bias=eps128[0:32, :])
                rex = pe.tile([32, GC], f32, tag="rexln")
                nc.scalar.activation(rex, lnv2, AF.Exp, scale=-0.5)
                psR2 = pp.tile([D, GC], f32, tag="ps_a", bufs=2, name="psR2")
                nc.tensor.matmul(psR2, lng_sb[l], rex[0:1, :],
                                 start=True, stop=True)
                y = pe.tile([D, GC], f32, tag="yln")
                nc.vector.tensor_mul(y, xsb, psR2)
                nc.vector.tensor_scalar(y, y, lnb_sb[l], None, OP.add)
                nc.vector.tensor_add(hid[l + 1], hid[l], y)
                if DEBUG_DUMPS:
                    dma(out=dbg[f"dbg_hid{l + 1}"], in_=hid[l + 1])
                    if l == 0:
                        dma(out=dbg["dbg_rden0"], in_=rden)
                        dma(out=dbg["dbg_xln0"], in_=xsb)
                        ktf = pe.tile([D, 512], f32, tag="ktf")
                        nc.vector.tensor_copy(ktf, KT[:, 0:512])
                        dma(out=dbg["dbg_kt0"], in_=ktf)

            # =============== FiLM + gene head ===============
            rep = pe.tile([D, GC], f32, tag="rep")
            nc.vector.tensor_scalar(rep, hid[L], film_a_sb, film_b_sb,
                                    OP.mult, OP.add)
            psH = pp.tile([64, GC], f32, tag="ps_a", bufs=2, name="psH")
            nc.tensor.matmul(psH, hw1_sb, rep, start=True, stop=True)
            hs = pe.tile([64, GC], f32, tag="hs")
            nc.scalar.activation(hs, psH, AF.Identity, bias=hb1_sb)
            sqh = pe.tile([64, GC], f32, tag="sqh")
            nc.scalar.activation(sqh, psH, AF.Square, bias=hb1_sb)
            psVh = pp.tile([32, GC], f32, tag="ps_a", bufs=2, name="psVh")
            nc.tensor.matmul(psVh, o64r, sqh, start=True, stop=True)
            lnvh = pe.tile([32, GC], f32, tag="lnvh")
            nc.scalar.activation(lnvh, psVh, AF.Ln, bias=eps128[0:32, :])
            rexh = pe.tile([32, GC], f32, tag="rexh")
            nc.scalar.activation(rexh, lnvh, AF.Exp, scale=-0.5)
            psRh = pp.tile([64, GC], f32, tag="ps_a", bufs=2, name="psRh")
            nc.tensor.matmul(psRh, hlng_sb, rexh[0:1, :], start=True, stop=True)
            yh = pe.tile([64, GC], f32, tag="yh")
            nc.vector.tensor_mul(yh, hs, psRh)
            hh = pe.tile([64, GC], f32, tag="hh")
            nc.scalar.activation(hh, yh, AF.Relu, bias=hlnb_sb)
            psp = pp.tile([1, GC], f32, tag="ps_a", bufs=2, name="psp")
            nc.tensor.matmul(psp, hw2_sb, hh, start=True, stop=True)
            pr = pe.tile([1, GC], f32, tag="pr")
            nc.scalar.activation(pr, psp, AF.Identity, bias=hb2_sb)
            dma(out=d_out, in_=pr)

    nc.compile()
    return nc


def _fold_inputs(inp):
    """Host-side exact folding of small per-batch vectors and LN
    centering into weights. Returns (shared, per_core) input dicts."""
    f = np.float32
    i = {k: np.asarray(v) for k, v in inp.items()}

    def colcenter(w):
        return (w - w.mean(axis=1, keepdims=True)).astype(f)

    def vcenter(b):
        return (b - b.mean()).astype(f)

    shared = {
        "w1c": colcenter(i["snp_w1"]),
        "b1c": vcenter(i["snp_b1"])[:, None],
        "ln1g": i["snp_ln1_g"].astype(f)[:, None],
        "ln1b": i["snp_ln1_b"].astype(f)[:, None],
        "w2c": colcenter(i["snp_w2"]),
        "b2c": vcenter(i["snp_b2"])[:, None],
        "hw1c": colcenter(i["head_w1"]),
        "hb1c": vcenter(i["head_b1"])[:, None],
        "hlng": i["head_ln_g"].astype(f)[None, :],
        "hlnb": i["head_ln_b"].astype(f)[:, None],
        "hw2": i["head_w2"].astype(f),
        "hb2": i["head_b2"].astype(f)[None, :],
    }
    g2 = i["snp_ln2_g"]
    b2 = i["snp_ln2_b"]
    for l in range(L):
        shared[f"wq{l}"] = i["wq"][l].astype(f)
        shared[f"wkg{l}"] = (g2[:, None] * i["wk"][l]).astype(f)
        shared[f"wvg{l}"] = (g2[:, None] * i["wv"][l]).astype(f)
        shared[f"woc{l}"] = colcenter(i["wo"][l]).astype(BF16 if ATT_BF16 else np.float32)
        shared[f"lng{l}"] = i["ln_g"][l].astype(f)[None, :]
        shared[f"lnb{l}"] = i["ln_b"][l].astype(f)[:, None]

    env_all = i["env_emb"][i["env_indices"].astype(np.int64)]  # [B, D]
    per_core = []
    for c in range(NCORES):
        b, j = c // 2, c % 2
        env = env_all[b]
        dq = env @ i["wq_env"]
        dk = env @ i["wk_env"]
        m = {
            "xT": np.concatenate(
                [i["genotypes"][b][None, :],
                 np.repeat(i["pcs"][b][:, None], S, axis=1)], 0).astype(f),
            "maskT": np.ascontiguousarray(
                i["cis_mask"].T[:, j * GC:(j + 1) * GC]).astype(BF16),
            "geneT": np.ascontiguousarray(
                i["gene_emb"][j * GC:(j + 1) * GC].T).astype(f),
            "film_a": (1.0 + np.tanh(env @ i["gamma_w"] + i["gamma_b"])
                       ).astype(f)[:, None],
            "film_b": (env @ i["beta_w"] + i["beta_b"]).astype(f)[:, None],
        }
        for l in range(L):
            cv = i["bv"][l] + b2 @ i["wv"][l]
            bo2 = i["bo"][l] + cv @ i["wo"][l]
            m[f"bqdq{l}"] = ((i["bq"][l] + dq) * SCALE).astype(f)[:, None]
            m[f"bkdk{l}"] = (i["bk"][l] + dk + b2 @ i["wk"][l]
                             ).astype(f)[:, None]
            m[f"bo2c{l}"] = vcenter(bo2)[:, None]
        per_core.append(m)
    return shared, per_core


def _get_program():
    if "nc" not in _CACHE:
        _CACHE["nc"] = _build_program()
    return _CACHE["nc"]


def run_cores(inp, trace=False):
    """Build in_maps, run on the 8 NeuronCores, return (preds, results)."""
    from concourse import bass_utils
    nc = _get_program()
    shared, per_core = _fold_inputs(inp)
    in_maps = [{**shared, **pc} for pc in per_core]
    res = bass_utils.run_bass_kernel_spmd(
        nc, in_maps, core_ids=list(range(NCORES)), trace=trace)
    preds = np.zeros((B, G), np.float32)
    for c in range(NCORES):
        b, j = c // 2, c % 2
        preds[b, j * GC:(j + 1) * GC] = res.results[c]["out"][0]
    return preds, res


def kernel(**inputs) -> np.ndarray:
    preds, _ = run_cores(inputs, trace=False)
    return preds


# revision 19
# speedup vs baseline: 1.8653x; 1.1310x over previous
"""Trainium2 Bass kernel for nn_EnvironmentConditionalTransformer.

Contract: kernel(**inputs) takes the FULL unsharded inputs (numpy arrays,
keys as in setup_inputs()) and returns the FULL [B, G] float32 output.

Sharding: 8 cores = batch(4) x gene-halves(2). Each core runs the SNP
encoder for its batch over all S=4096 SNPs, projects K/V, and computes
masked attention + FiLM + head for its 512 genes. Zero collectives; the
host concatenates the per-core [1, 512] outputs.

Math notes (all exact reformulations, validated vs reference to ~1e-6):
  * softmax(where(mask, s, -1e9)) == exp(s)*mask / sum(exp(s)*mask)
    because |s| < 1 here (no max-subtraction needed) and exp(-1e9) == 0.
  * LayerNorm mean-centering is folded into the producing weight matrix
    (W - colmean(W)), so on-device LNs are variance-only.
  * LN2's affine (gain/bias) is folded into wk/wv and downstream biases.
  * V bias and the ones-normalization are folded into bo via softmax
    rows summing to 1 (denominator handled explicitly).
  * rsqrt(v+eps) = exp(-0.5*ln(v+eps)) keeps ACT in one table set (exp).
"""

import sys

if "/opt/trn_rl_repo" not in sys.path:
    sys.path.insert(0, "/opt/trn_rl_repo")

import numpy as np
import ml_dtypes

B, S, G, P, E, D, H, L = 4, 4096, 1024, 20, 3, 128, 4, 2
DK = D // H          # 32
GC = G // 2          # 512 genes per core
NCORES = 8
EPS = 1e-5
SCALE = 1.0 / float(np.sqrt(np.float32(DK)))
NCHUNK = S // 512    # 8
NT = S // 128        # 32 s-tiles

_CACHE = {}

BF16 = ml_dtypes.bfloat16
# Attention-path precision: True = bf16 attn weights/mask/V (fast DVE),
# False = all-f32 (accuracy reference).
ATT_BF16 = True
import os as _os
DEBUG_DUMPS = bool(_os.environ.get("KERNEL_DEBUG_DUMPS"))


def _build_program():
    import concourse.bass as bass
    import concourse.bacc as bacc
    import concourse.tile as tile
    from concourse import mybir

    # All ACT funcs used here (Exp/Ln/Square/Relu/Identity/Copy) live in
    # the natural_log_exp_and_others table set; blank the others so the
    # table-load placement picks one set once instead of ping-ponging.
    import concourse.hw_specs as hw_specs
    if not getattr(bacc, "_act_tables_patched", False):
        _orig_gat = hw_specs.get_activation_tables

        def _gat_one_set(arch):
            t = _orig_gat(arch)
            return {k: (v if k == "natural_log_exp_and_others" else set())
                    for k, v in t.items()}

        bacc.get_activation_tables = _gat_one_set
        bacc._act_tables_patched = True

    f32 = mybir.dt.float32
    bf16 = mybir.dt.bfloat16
    at_dt = bf16 if ATT_BF16 else f32
    AF = mybir.ActivationFunctionType
    OP = mybir.AluOpType

    nc = bacc.Bacc("TRN2", debug=False, target_bir_lowering=False,
                   num_devices=NCORES)

    def din(name, shape, dt=f32):
        return nc.dram_tensor(name, list(shape), dt, kind="ExternalInput").ap()

    # ---- per-core data (differs across cores) ----
    d_xT = din("xT", [21, S], at_dt)                    # [geno; pcs-rows] transposed
    d_maskT = din("maskT", [S, GC], bf16)        # cis mask^T slice, 0/1
    d_geneT = din("geneT", [D, GC])              # gene_emb^T slice
    d_film_a = din("film_a", [D, 1])             # 1 + tanh(env@gamma_w+gb)
    d_film_b = din("film_b", [D, 1])             # env@beta_w + bb
    d_bqdq = [din(f"bqdq{l}", [D, 1]) for l in range(L)]   # (bq+dq)*SCALE
    d_bkdk = [din(f"bkdk{l}", [D, 1]) for l in range(L)]   # bk+dk+ln2b@wk
    d_bo2c = [din(f"bo2c{l}", [D, 1]) for l in range(L)]   # centered bo''

    # ---- shared weights ----
    d_w1 = din("w1c", [21, 64], at_dt)
    d_b1 = din("b1c", [64, 1])
    d_ln1g = din("ln1g", [64, 1])
    d_ln1b = din("ln1b", [64, 1])
    d_w2 = din("w2c", [64, D], at_dt)
    d_b2 = din("b2c", [D, 1])
    d_wq = [din(f"wq{l}", [D, D]) for l in range(L)]
    d_wk = [din(f"wkg{l}", [D, D], at_dt) for l in range(L)]
    d_wv = [din(f"wvg{l}", [D, D], at_dt) for l in range(L)]
    d_wo = [din(f"woc{l}", [D, D], at_dt) for l in range(L)]
    d_lng = [din(f"lng{l}", [1, D]) for l in range(L)]
    d_lnb = [din(f"lnb{l}", [D, 1]) for l in range(L)]
    d_hw1 = din("hw1c", [D, 64])
    d_hb1 = din("hb1c", [64, 1])
    d_hlng = din("hlng", [1, 64])
    d_hlnb = din("hlnb", [64, 1])
    d_hw2 = din("hw2", [64, 1])
    d_hb2 = din("hb2", [1, 1])

    d_out = nc.dram_tensor("out", [1, GC], f32, kind="ExternalOutput").ap()
    dbg = {}
    if DEBUG_DUMPS:
        for nm, shp in [("dbg_h", [64, S]), ("dbg_sf", [D, S]),
                        ("dbg_hid1", [D, GC]), ("dbg_hid2", [D, GC]),
                        ("dbg_xln0", [D, GC]), ("dbg_rden0", [128, GC]),
                        ("dbg_kt0", [D, 512]), ("dbg_qt0", [D, GC])]:
            dbg[nm] = nc.dram_tensor(nm, shp, f32, kind="ExternalOutput").ap()

    with tile.TileContext(nc) as tc:
        with (
            tc.tile_pool(name="pw", bufs=1) as pw,          # persistent sbuf
            tc.tile_pool(name="pe", bufs=1) as pe,          # working sbuf
            tc.tile_pool(name="pa", bufs=3 if ATT_BF16 else 2) as pa,
            tc.tile_pool(name="pp", bufs=1, space="PSUM") as pp,
        ):
            dma = nc.sync.dma_start

            # ---------------- constants ----------------
            ones_col = pw.tile([128, 128], f32, tag="ones_col")
            nc.vector.memset(ones_col, 1.0)
            o64r = pw.tile([64, 32], f32, tag="o64r")       # 1/64 stats lhsT
            nc.vector.memset(o64r, 1.0 / 64.0)
            o128r = pw.tile([128, 32], f32, tag="o128r")    # 1/128 stats lhsT
            nc.vector.memset(o128r, 1.0 / 128.0)
            ones_bf = pw.tile([128, 1], at_dt, tag="ones_bf")
            nc.vector.memset(ones_bf, 1.0)
            eps128 = pw.tile([128, 1], f32, tag="eps128")
            nc.vector.memset(eps128, EPS)
            zeros128 = pw.tile([128, 128], at_dt, tag="zeros128")
            nc.vector.memset(zeros128, 0.0)

            # ---------------- load inputs ----------------
            x_sb = pw.tile([21, S], at_dt, tag="xT")
            dma(out=x_sb, in_=d_xT)
            w1_sb = pw.tile([21, 64], at_dt, tag="w1")
            dma(out=w1_sb, in_=d_w1)
            b1_sb = pw.tile([64, 1], f32, tag="b1")
            dma(out=b1_sb, in_=d_b1)
            ln1g_sb = pw.tile([64, 1], f32, tag="ln1g")
            dma(out=ln1g_sb, in_=d_ln1g)
            ln1b_sb = pw.tile([64, 1], f32, tag="ln1b")
            dma(out=ln1b_sb, in_=d_ln1b)
            w2_sb = pw.tile([64, D], at_dt, tag="w2")
            dma(out=w2_sb, in_=d_w2)
            b2_sb = pw.tile([D, 1], f32, tag="b2")
            dma(out=b2_sb, in_=d_b2)

            mask_sb = pw.tile([128, NT, GC], bf16, tag="mask")
            m_re = d_maskT.rearrange("(t p) g -> p t g", p=128)
            for q in range(4):
                dma(out=mask_sb[:, q * 8:(q + 1) * 8, :],
                    in_=m_re[:, q * 8:(q + 1) * 8, :])

            hid = [pw.tile([D, GC], f32, tag=f"hid{i}", name=f"hid{i}")
                   for i in range(L + 1)]
            dma(out=hid[0], in_=d_geneT)
            film_a_sb = pw.tile([D, 1], f32, tag="film_a")
            dma(out=film_a_sb, in_=d_film_a)
            film_b_sb = pw.tile([D, 1], f32, tag="film_b")
            dma(out=film_b_sb, in_=d_film_b)

            wq_sb, wk_sb, wv_sb, wo_sb = [], [], [], []
            bqdq_sb, bkdk_sb, bo2c_sb, lng_sb, lnb_sb = [], [], [], [], []
            for l in range(L):
                t_ = pw.tile([D, D], f32, tag=f"wq{l}")
                dma(out=t_, in_=d_wq[l]); wq_sb.append(t_)
                t_ = pw.tile([D, D], at_dt, tag=f"wk{l}")
                dma(out=t_, in_=d_wk[l]); wk_sb.append(t_)
                t_ = pw.tile([D, D], at_dt, tag=f"wv{l}")
                dma(out=t_, in_=d_wv[l]); wv_sb.append(t_)
                t_ = pw.tile([D, D], at_dt, tag=f"wo{l}")
                dma(out=t_, in_=d_wo[l]); wo_sb.append(t_)
                t_ = pw.tile([D, 1], f32, tag=f"bqdq{l}")
                dma(out=t_, in_=d_bqdq[l]); bqdq_sb.append(t_)
                t_ = pw.tile([D, 1], f32, tag=f"bkdk{l}")
                dma(out=t_, in_=d_bkdk[l]); bkdk_sb.append(t_)
                t_ = pw.tile([D, 1], f32, tag=f"bo2c{l}")
                dma(out=t_, in_=d_bo2c[l]); bo2c_sb.append(t_)
                t_ = pw.tile([1, D], f32, tag=f"lng{l}")
                dma(out=t_, in_=d_lng[l]); lng_sb.append(t_)
                t_ = pw.tile([D, 1], f32, tag=f"lnb{l}")
                dma(out=t_, in_=d_lnb[l]); lnb_sb.append(t_)
            hw1_sb = pw.tile([D, 64], f32, tag="hw1")
            dma(out=hw1_sb, in_=d_hw1)
            hb1_sb = pw.tile([64, 1], f32, tag="hb1")
            dma(out=hb1_sb, in_=d_hb1)
            hlng_sb = pw.tile([1, 64], f32, tag="hlng")
            dma(out=hlng_sb, in_=d_hlng)
            hlnb_sb = pw.tile([64, 1], f32, tag="hlnb")
            dma(out=hlnb_sb, in_=d_hlnb)
            hw2_sb = pw.tile([64, 1], f32, tag="hw2")
            dma(out=hw2_sb, in_=d_hw2)
            hb2_sb = pw.tile([1, 1], f32, tag="hb2")
            dma(out=hb2_sb, in_=d_hb2)

            # =============== SNP encoder (feature-major) ===============
            # LN1: h = relu(((x@w1c + b1c) * r1) * g1 + b1_ln)
            h_sb = pw.tile([64, S], at_dt, tag="h1")
            psV1 = pp.tile([128, 1024], f32, tag="ps_b", bufs=1,
                           name="psV1")
            for c in range(NCHUNK):
                sl = slice(c * 512, (c + 1) * 512)
                rg = 32 * (c % 4)
                psA = pp.tile([64, 512], f32, tag="ps_a", bufs=2, name="psA")
                nc.tensor.matmul(psA, w1_sb, x_sb[:, sl], start=True, stop=True)
                sqA = pe.tile([64, 512], f32, tag="sqA", bufs=3)
                nc.scalar.activation(sqA, psA, AF.Square, bias=b1_sb)
                nc.tensor.matmul(
                    psV1[rg:rg + 32, (c // 4) * 512:(c // 4 + 1) * 512],
                    o64r, sqA, start=True, stop=True, tile_position=(0, rg))
                nc.vector.tensor_scalar(h_sb[:, sl], psA, b1_sb, None, OP.add)
            rex1 = [None, None]
            for g_ in range(2):
                lnv = pe.tile([128, 512], f32, tag="lnv", bufs=2)
                nc.scalar.activation(lnv, psV1[:, g_ * 512:(g_ + 1) * 512],
                                     AF.Ln, bias=eps128)
                rex1[g_] = pe.tile([128, 512], f32, tag=f"rex1{g_}", bufs=1,
                                   name=f"rex1{g_}")
                nc.scalar.activation(rex1[g_], lnv, AF.Exp, scale=-0.5)
            for c in range(NCHUNK):
                sl = slice(c * 512, (c + 1) * 512)
                rg = 32 * (c % 4)
                psR = pp.tile([64, 512], f32, tag="ps_a", bufs=2, name="psR1")
                nc.tensor.matmul(psR, ones_col[rg:rg + 1, 0:64],
                                 rex1[c // 4][rg:rg + 1, :],
                                 start=True, stop=True, tile_position=(rg, 0))
                nc.vector.tensor_mul(h_sb[:, sl], h_sb[:, sl], psR)
                nc.scalar.activation(h_sb[:, sl], h_sb[:, sl], AF.Relu,
                                     bias=ln1b_sb, scale=ln1g_sb)

            # LN2 (affine folded into wk/wv): sf = (h@w2c + b2c) * r2
            sf_sb = pw.tile([D, S], at_dt, tag="snpf")
            psV2 = pp.tile([128, 1024], f32, tag="ps_b", bufs=1,
                           name="psV2")
            for c in range(NCHUNK):
                sl = slice(c * 512, (c + 1) * 512)
                rg = 32 * (c % 4)
                psB = pp.tile([128, 512], f32, tag="ps_a", bufs=2, name="psB")
                nc.tensor.matmul(psB, w2_sb, h_sb[:, sl], start=True, stop=True)
                sqB = pe.tile([128, 512], f32, tag="sqB", bufs=3)
                nc.scalar.activation(sqB, psB, AF.Square, bias=b2_sb)
                nc.tensor.matmul(
                    psV2[rg:rg + 32, (c // 4) * 512:(c // 4 + 1) * 512],
                    o128r, sqB, start=True, stop=True, tile_position=(0, rg))
                nc.vector.tensor_scalar(sf_sb[:, sl], psB, b2_sb, None, OP.add)
            rex2 = [None, None]
            for g_ in range(2):
                lnv = pe.tile([128, 512], f32, tag="lnv", bufs=2)
                nc.scalar.activation(lnv, psV2[:, g_ * 512:(g_ + 1) * 512],
                                     AF.Ln, bias=eps128)
                rex2[g_] = pe.tile([128, 512], f32, tag=f"rex2{g_}", bufs=1,
                                   name=f"rex2{g_}")
                nc.scalar.activation(rex2[g_], lnv, AF.Exp, scale=-0.5)
            for c in range(NCHUNK):
                sl = slice(c * 512, (c + 1) * 512)
                rg = 32 * (c % 4)
                psR = pp.tile([128, 512], f32, tag="ps_a", bufs=2, name="psR2e")
                nc.tensor.matmul(psR, ones_col[rg:rg + 1, 0:128],
                                 rex2[c // 4][rg:rg + 1, :],
                                 start=True, stop=True, tile_position=(rg, 0))
                nc.vector.tensor_mul(sf_sb[:, sl], sf_sb[:, sl], psR)

            if DEBUG_DUMPS:
                nc.gpsimd.dma_start(out=dbg["dbg_h"], in_=h_sb)
                nc.gpsimd.dma_start(out=dbg["dbg_sf"], in_=sf_sb)

            # =============== transformer layers ===============
            for l in range(L):
                # Q^T [D, GC] bf16, pre-scaled by 1/sqrt(dk)
                psQ = pp.tile([D, GC], f32, tag="ps_a", bufs=2, name="psQ")
                nc.tensor.matmul(psQ, wq_sb[l], hid[l], start=True, stop=True)
                QT = pe.tile([D, GC], at_dt, tag="QT")
                nc.scalar.activation(QT, psQ, AF.Identity,
                                     bias=bqdq_sb[l], scale=SCALE)
                # K^T [D, S] bf16
                KT = pe.tile([D, S], at_dt, tag="KT")
                for c in range(NCHUNK):
                    sl = slice(c * 512, (c + 1) * 512)
                    psK = pp.tile([D, 512], f32, tag="ps_a", bufs=2, name="psK")
                    nc.tensor.matmul(psK, wk_sb[l], sf_sb[:, sl],
                                     start=True, stop=True)
                    nc.vector.tensor_scalar(KT[:, sl], psK, bkdk_sb[l],
                                            None, OP.add)
                if DEBUG_DUMPS and l == 0:
                    qtf = pe.tile([D, GC], f32, tag="qtf")
                    nc.vector.tensor_copy(qtf, QT)
                    dma(out=dbg["dbg_qt0"], in_=qtf)
                # V [s-local, (t, dv)] bf16
                Vb = pe.tile([128, S], at_dt, tag="Vb")
                for q in range(NCHUNK):
                    psv = pp.tile([128, 512], f32, tag="ps_a", bufs=2, name="psv")
                    for k in range(4):
                        t = 4 * q + k
                        nc.tensor.matmul(psv[:, k * 128:(k + 1) * 128],
                                         sf_sb[:, t * 128:(t + 1) * 128],
                                         wv_sb[l], start=True, stop=True)
                    nc.vector.tensor_copy(Vb[:, q * 512:(q + 1) * 512], psv)

                # attention: scores^T -> exp -> mask -> attn@V (+denoms)
                # psOD bank h: attnV out for head h at partitions [32h,32h+32)
                # and its denominator row at partition 32*((h+1)%4).
                psOD = pp.tile([128, 4 * 512], f32, tag="ps_b", bufs=1, name="psOD")
                for h in range(H):
                    nc.tensor.matmul(psOD[:, h * 512:(h + 1) * 512],
                                     zeros128, sf_sb[:, 0:512],
                                     start=True, stop=False,
                                     skip_group_check=True)
                for t in range(NT):
                    at = pa.tile([128, 4 * 512], at_dt, tag="attn")
                    for hp in range(2):
                        psS = pp.tile([128, 2 * 512], f32, tag="ps_a",
                                      bufs=2, name="psS")
                        for hh in range(2):
                            h = 2 * hp + hh
                            nc.tensor.matmul(
                                psS[:, hh * 512:(hh + 1) * 512],
                                KT[32 * h:32 * h + 32, t * 128:(t + 1) * 128],
                                QT[32 * h:32 * h + 32, :],
                                start=True, stop=True,
                                tile_position=(32 * h, 0))
                        nc.scalar.activation(
                            at[:, hp * 1024:(hp + 1) * 1024], psS, AF.Exp)
                    at3 = at.rearrange("p (h g) -> p h g", h=4)
                    mt = mask_sb[:, t, :]
                    m_bc = bass.AP(tensor=mt.tensor, offset=mt.offset,
                                   ap=[list(mt.ap[0]), [0, 4], list(mt.ap[1])])
                    nc.vector.tensor_mul(at3, at3, m_bc)
                    for h in range(H):
                        nc.tensor.matmul(
                            psOD[32 * h:32 * h + 32, h * 512:(h + 1) * 512],
                            Vb[:, t * 128 + 32 * h:t * 128 + 32 * h + 32],
                            at[:, h * 512:(h + 1) * 512],
                            start=False, stop=False,
                            tile_position=(0, 32 * h),
                            skip_group_check=True)
                    for h in range(H):
                        jh = 32 * ((h + 1) % 4)
                        nc.tensor.matmul(
                            psOD[jh:jh + 1, h * 512:(h + 1) * 512],
                            ones_bf,
                            at[:, h * 512:(h + 1) * 512],
                            start=False, stop=(t == NT - 1),
                            tile_position=(0, jh),
                            skip_group_check=True)

                # normalize by denominators, project, LN, residual
                lden = pe.tile([128, GC], f32, tag="rden")
                if DEBUG_DUMPS:
                    nc.vector.memset(lden, 0.0)
                for h in range(H):
                    jh = 32 * ((h + 1) % 4)
                    nc.scalar.activation(lden[jh:jh + 1, :],
                                         psOD[jh:jh + 1, h * 512:(h + 1) * 512],
                                         AF.Ln)
                psRb = pp.tile([128, GC], f32, tag="ps_a", bufs=2, name="psRb")
                for h in range(H):
                    jh = 32 * ((h + 1) % 4)
                    nc.tensor.matmul(psRb[32 * h:32 * h + 32, :],
                                     ones_col[jh:jh + 1, 0:32],
                                     lden[jh:jh + 1, :],
                                     start=True, stop=True,
                                     tile_position=(jh, 32 * h))
                rbc = pe.tile([128, GC], f32, tag="rbc_sb")
                nc.scalar.activation(rbc, psRb, AF.Exp, scale=-1.0)
                outn = pe.tile([128, GC], at_dt, tag="outn")
                for h in range(H):
                    hp = slice(32 * h, 32 * h + 32)
                    nc.vector.tensor_mul(outn[hp, :],
                                         psOD[hp, h * 512:(h + 1) * 512],
                                         rbc[hp, :])

                psP = pp.tile([D, GC], f32, tag="ps_a", bufs=2, name="psP")
                nc.tensor.matmul(psP, wo_sb[l], outn, start=True, stop=True)
                xsb = pe.tile([D, GC], f32, tag="xln")
                nc.scalar.activation(xsb, psP, AF.Identity, bias=bo2c_sb[l])
                sq = pe.tile([D, GC], f32, tag="sqln")
                nc.scalar.activation(sq, psP, AF.Square, bias=bo2c_sb[l])
                psVr = pp.tile([32, GC], f32, tag="ps_a", bufs=2, name="psVr")
                nc.tensor.matmul(psVr, o128r, sq, start=True, stop=True)
                lnv2 = pe.tile([32, GC], f32, tag="lnv2")
                nc.scalar.activation(lnv2, psVr, AF.Ln, bias=eps128[0:32, :])
                rex = pe.tile([32, GC], f32, tag="rexln")
                nc.scalar.activation(rex, lnv2, AF.Exp, scale=-0.5)
                psR2 = pp.tile([D, GC], f32, tag="ps_a", bufs=2, name="psR2")
                nc.tensor.matmul(psR2, lng_sb[l], rex[0:1, :],
                                 start=True, stop=True)
                y = pe.tile([D, GC], f32, tag="yln")
                nc.vector.tensor_mul(y, xsb, psR2)
                nc.vector.tensor_scalar(y, y, lnb_sb[l], None, OP.add)
                nc.vector.tensor_add(hid[l + 1], hid[l], y)
                if DEBUG_DUMPS:
                    dma(out=dbg[f"dbg_hid{l + 1}"], in_=hid[l + 1])
                    if l == 0:
                        dma(out=dbg["dbg_rden0"], in_=rbc)
                        dma(out=dbg["dbg_xln0"], in_=xsb)
                        ktf = pe.tile([D, 512], f32, tag="ktf")
                        nc.vector.tensor_copy(ktf, KT[:, 0:512])
                        dma(out=dbg["dbg_kt0"], in_=ktf)

            # =============== FiLM + gene head ===============
            rep = pe.tile([D, GC], f32, tag="rep")
            nc.vector.tensor_scalar(rep, hid[L], film_a_sb, film_b_sb,
                                    OP.mult, OP.add)
            psH = pp.tile([64, GC], f32, tag="ps_a", bufs=2, name="psH")
            nc.tensor.matmul(psH, hw1_sb, rep, start=True, stop=True)
            hs = pe.tile([64, GC], f32, tag="hs")
            nc.scalar.activation(hs, psH, AF.Identity, bias=hb1_sb)
            sqh = pe.tile([64, GC], f32, tag="sqh")
            nc.scalar.activation(sqh, psH, AF.Square, bias=hb1_sb)
            psVh = pp.tile([32, GC], f32, tag="ps_a", bufs=2, name="psVh")
            nc.tensor.matmul(psVh, o64r, sqh, start=True, stop=True)
            lnvh = pe.tile([32, GC], f32, tag="lnvh")
            nc.scalar.activation(lnvh, psVh, AF.Ln, bias=eps128[0:32, :])
            rexh = pe.tile([32, GC], f32, tag="rexh")
            nc.scalar.activation(rexh, lnvh, AF.Exp, scale=-0.5)
            psRh = pp.tile([64, GC], f32, tag="ps_a", bufs=2, name="psRh")
            nc.tensor.matmul(psRh, hlng_sb, rexh[0:1, :], start=True, stop=True)
            yh = pe.tile([64, GC], f32, tag="yh")
            nc.vector.tensor_mul(yh, hs, psRh)
            hh = pe.tile([64, GC], f32, tag="hh")
            nc.scalar.activation(hh, yh, AF.Relu, bias=hlnb_sb)
            psp = pp.tile([1, GC], f32, tag="ps_a", bufs=2, name="psp")
            nc.tensor.matmul(psp, hw2_sb, hh, start=True, stop=True)
            pr = pe.tile([1, GC], f32, tag="pr")
            nc.scalar.activation(pr, psp, AF.Identity, bias=hb2_sb)
            dma(out=d_out, in_=pr)

    nc.compile()
    return nc


def _fold_inputs(inp):
    """Host-side exact folding of small per-batch vectors and LN
    centering into weights. Returns (shared, per_core) input dicts."""
    f = np.float32
    i = {k: np.asarray(v) for k, v in inp.items()}

    def colcenter(w):
        return (w - w.mean(axis=1, keepdims=True)).astype(f)

    def vcenter(b):
        return (b - b.mean()).astype(f)

    at_np = BF16 if ATT_BF16 else np.float32
    shared = {
        "w1c": colcenter(i["snp_w1"]).astype(at_np),
        "b1c": vcenter(i["snp_b1"])[:, None],
        "ln1g": i["snp_ln1_g"].astype(f)[:, None],
        "ln1b": i["snp_ln1_b"].astype(f)[:, None],
        "w2c": colcenter(i["snp_w2"]).astype(at_np),
        "b2c": vcenter(i["snp_b2"])[:, None],
        "hw1c": colcenter(i["head_w1"]),
        "hb1c": vcenter(i["head_b1"])[:, None],
        "hlng": i["head_ln_g"].astype(f)[None, :],
        "hlnb": i["head_ln_b"].astype(f)[:, None],
        "hw2": i["head_w2"].astype(f),
        "hb2": i["head_b2"].astype(f)[None, :],
    }
    g2 = i["snp_ln2_g"]
    b2 = i["snp_ln2_b"]
    for l in range(L):
        shared[f"wq{l}"] = i["wq"][l].astype(f)
        shared[f"wkg{l}"] = (g2[:, None] * i["wk"][l]).astype(at_np)
        shared[f"wvg{l}"] = (g2[:, None] * i["wv"][l]).astype(at_np)
        shared[f"woc{l}"] = colcenter(i["wo"][l]).astype(BF16 if ATT_BF16 else np.float32)
        shared[f"lng{l}"] = i["ln_g"][l].astype(f)[None, :]
        shared[f"lnb{l}"] = i["ln_b"][l].astype(f)[:, None]

    env_all = i["env_emb"][i["env_indices"].astype(np.int64)]  # [B, D]
    per_core = []
    for c in range(NCORES):
        b, j = c // 2, c % 2
        env = env_all[b]
        dq = env @ i["wq_env"]
        dk = env @ i["wk_env"]
        m = {
            "xT": np.concatenate(
                [i["genotypes"][b][None, :],
                 np.repeat(i["pcs"][b][:, None], S, axis=1)], 0).astype(at_np),
            "maskT": np.ascontiguousarray(
                i["cis_mask"].T[:, j * GC:(j + 1) * GC]).astype(BF16),
            "geneT": np.ascontiguousarray(
                i["gene_emb"][j * GC:(j + 1) * GC].T).astype(f),
            "film_a": (1.0 + np.tanh(env @ i["gamma_w"] + i["gamma_b"])
                       ).astype(f)[:, None],
            "film_b": (env @ i["beta_w"] + i["beta_b"]).astype(f)[:, None],
        }
        for l in range(L):
            cv = i["bv"][l] + b2 @ i["wv"][l]
            bo2 = i["bo"][l] + cv @ i["wo"][l]
            m[f"bqdq{l}"] = ((i["bq"][l] + dq) * SCALE).astype(f)[:, None]
            m[f"bkdk{l}"] = (i["bk"][l] + dk + b2 @ i["wk"][l]
                             ).astype(f)[:, None]
            m[f"bo2c{l}"] = vcenter(bo2)[:, None]
        per_core.append(m)
    return shared, per_core


def _get_program():
    if "nc" not in _CACHE:
        _CACHE["nc"] = _build_program()
    return _CACHE["nc"]


def run_cores(inp, trace=False):
    """Build in_maps, run on the 8 NeuronCores, return (preds, results)."""
    from concourse import bass_utils
    nc = _get_program()
    shared, per_core = _fold_inputs(inp)
    in_maps = [{**shared, **pc} for pc in per_core]
    res = bass_utils.run_bass_kernel_spmd(
        nc, in_maps, core_ids=list(range(NCORES)), trace=trace)
    preds = np.zeros((B, G), np.float32)
    for c in range(NCORES):
        b, j = c // 2, c % 2
        preds[b, j * GC:(j + 1) * GC] = res.results[c]["out"][0]
    return preds, res


def kernel(**inputs) -> np.ndarray:
    preds, _ = run_cores(inputs, trace=False)
    return preds
